# revision 1
# baseline (speedup 1.0000x reference)
"""Builder for the GIN graph-classifier Bass kernel (8-core SPMD TRN2).

Layout/contract notes:
- N nodes padded to NPAD = 8*49*128 = 50176; core c owns dst nodes
  [c*6272, (c+1)*6272), i.e. 49 blocks of 128 nodes.
- Edges assigned to the core owning dst. Within a core, edges are grouped
  by (block, parity(src)), sorted by src, padded per (block,parity) to
  T_P tiles of 128 edges (idx pad = previous idx, dstrel pad = -1).
- Gather groups: blocks processed in groups (GRP_BLOCKS at a time); per
  (group,parity) a single dma_gather fetches all that group's tiles.
- Tile t of block b holds 128 edges; M_t[e, n] = (dstrel[e] == n) is built
  with one DVE tensor_scalar(is_equal) against a host-provided iota row.
- agg_nm[128n, D] += M_t.T @ g_t accumulated in PSUM over the block tiles.
- z = agg + own; transpose chunks via PE; MLP on fp16 weights; conv1 h1
  shards are AllGathered in fp16 to form the conv2 gather table.
- Output: per-core partial graph sums [64, 256] f32; host divides by
  counts and applies the classifier.
"""
import numpy as np

import concourse.bass as bass
import concourse.mybir as mybir
import concourse.tile as tile
from concourse import bacc

F32 = mybir.dt.float32
F16 = mybir.dt.float16
I16 = mybir.dt.int16

NCORES = 8
BLK = 128
NB = 49                      # blocks per core
SH = NB * BLK                # 6272 nodes per core shard
NPAD = NCORES * SH           # 50176
G = 64                       # graphs
HID = 256
IN_DIM = 128
RELU = mybir.ActivationFunctionType.Relu
AF_COPY = mybir.ActivationFunctionType.Copy


def preprocess(x, edge_index, batch, weights, grp_blocks=2):
    """numpy preprocessing -> per-core input maps + metadata dict."""
    N = x.shape[0]
    src = np.asarray(edge_index[0], dtype=np.int64)
    dst = np.asarray(edge_index[1], dtype=np.int64)
    batch = np.asarray(batch, dtype=np.int64)

    core = dst // SH
    blk = (dst % SH) // BLK
    dst_rel = dst % BLK
    par = src % 2
    half = src // 2

    # global sort: (core, blk, parity, src) for gather locality
    order = np.lexsort((src, par, blk, core))
    core_s, blk_s, par_s = core[order], blk[order], par[order]
    half_s, dstrel_s = half[order], dst_rel[order]

    # tiles per (core, blk, parity)
    key = (core_s * NB + blk_s) * 2 + par_s
    counts = np.bincount(key, minlength=NCORES * NB * 2)
    T_P = int(np.ceil(counts.max() / BLK))
    cap = T_P * BLK

    # build padded idx (int16 halves) and dstrel (f32) arrays
    # padded layout per core: blocks in order; per block: even tiles then odd
    n_cell = NCORES * NB * 2
    idx_pad = np.zeros((n_cell, cap), dtype=np.int16)
    dr_pad = np.full((n_cell, cap), -1.0, dtype=np.float32)
    cell_starts = np.concatenate([[0], np.cumsum(counts)])
    for cell in range(n_cell):
        s, e = cell_starts[cell], cell_starts[cell + 1]
        cnt = e - s
        idx_pad[cell, :cnt] = half_s[s:e].astype(np.int16)
        dr_pad[cell, :cnt] = dstrel_s[s:e].astype(np.float32)
        if cnt < cap:
            fill = half_s[e - 1] if cnt > 0 else 0
            idx_pad[cell, cnt:] = fill

    idx_pad = idx_pad.reshape(NCORES, NB, 2, cap)
    dr_pad = dr_pad.reshape(NCORES, NB, 2, cap)

    # groups of blocks
    groups = [list(range(i, min(i + grp_blocks, NB)))
              for i in range(0, NB, grp_blocks)]

    # per-core flattened gather idx tensor: for each (group, parity), a
    # contiguous run of gb*cap int16 indices, wrapped into 16 partitions
    # (idx i -> [i % 16, i // 16]) and replicated to 128 partitions.
    idx_cols_per = [len(g) * cap // 16 for g in groups for _ in (0, 1)]
    total_idx_cols = sum(idx_cols_per)
    idx_flat = np.zeros((NCORES, 128, total_idx_cols), dtype=np.int16)
    col_off = []
    for c in range(NCORES):
        off = 0
        for gi, g in enumerate(groups):
            for p in (0, 1):
                lst = idx_pad[c, g, p, :].reshape(-1)  # gb*cap
                w = lst.reshape(-1, 16).T               # [16, gb*cap/16]
                cols = w.shape[1]
                idx_flat[c, :, off:off + cols] = np.tile(w, (8, 1))
                if c == 0:
                    col_off.append((gi, p, off, cols))
                off += cols

    # dstrel tensor: [core, 128, NT] where NT = NB*2*T_P; tile t of
    # (blk b, parity p) at column b*2*T_P + p*T_P + t
    NT = NB * 2 * T_P
    dr_t = dr_pad.reshape(NCORES, NB * 2 * T_P, BLK).transpose(0, 2, 1).copy()

    # pooling selector S: [core, 128, NB*64]
    S = np.zeros((NCORES, BLK, NB * G), dtype=np.float32)
    node = np.arange(NPAD)
    valid = node < N
    b_of = np.where(valid, np.concatenate([batch, np.zeros(NPAD - N, np.int64)]), -1)
    for c in range(NCORES):
        for b in range(NB):
            rows = b_of[c * SH + b * BLK: c * SH + (b + 1) * BLK]
            for p in range(BLK):
                if rows[p] >= 0:
                    S[c, p, b * G + rows[p]] = 1.0

    x_pad = np.zeros((NPAD, IN_DIM), dtype=np.float32)
    x_pad[:N] = np.asarray(x, dtype=np.float32)

    W1a, b1a, W1b, b1b, W2a, b2a, W2b, b2b = weights
    wpack = {
        "W1aT": np.ascontiguousarray(W1a.T.astype(np.float16)),          # [128, 256]
        "W1bT": np.ascontiguousarray(                                     # [128, 2*256]
            W1b.T.reshape(2, 128, 256).transpose(1, 0, 2).reshape(128, 512)),
        "W2aT": np.ascontiguousarray(
            W2a.T.reshape(2, 128, 256).transpose(1, 0, 2).reshape(128, 512)).astype(np.float16),
        "W2bT": np.ascontiguousarray(
            W2b.T.reshape(2, 128, 256).transpose(1, 0, 2).reshape(128, 512)).astype(np.float16),
        "bA1": np.ascontiguousarray(b1a.reshape(2, 128).T.astype(np.float32)),  # [128,2]
        "bA2": np.ascontiguousarray(b2a.reshape(2, 128).T.astype(np.float32)),
        "bB1": np.tile(b1b.astype(np.float32)[None, :], (128, 1)),       # [128,256]
        "bB2": np.tile(b2b.astype(np.float32)[None, :], (128, 1)),
    }
    wpack["W1bT"] = wpack["W1bT"].astype(np.float16)

    iota = np.tile(np.arange(BLK, dtype=np.float32)[None, :], (BLK, 1))
    ident = np.eye(BLK, dtype=np.float32)

    in_maps = []
    for c in range(NCORES):
        m = {
            "x_tab": x_pad,
            "x_own": np.ascontiguousarray(x_pad[c * SH:(c + 1) * SH]),
            "idx": idx_flat[c],
            "dstrel": dr_t[c],
            "Ssel": S[c],
            "iota": iota, "ident": ident,
        }
        m.update(wpack)
        in_maps.append(m)

    meta = dict(T_P=T_P, NT=NT, groups=groups, col_off=col_off,
                total_idx_cols=total_idx_cols, cap=cap)
    return in_maps, meta


def build(meta, fp32_conv1_mm=True):
    """Build the Bass program. Returns nc."""
    T_P = meta["T_P"]
    NT = meta["NT"]
    groups = meta["groups"]
    col_off = meta["col_off"]
    cap = meta["cap"]

    nc = bacc.Bacc("TRN2", target_bir_lowering=False, debug=False,
                   num_devices=NCORES)

    x_tab = nc.dram_tensor("x_tab", [NPAD, IN_DIM], F32, kind="ExternalInput")
    x_own = nc.dram_tensor("x_own", [SH, IN_DIM], F32, kind="ExternalInput")
    idx_in = nc.dram_tensor("idx", [128, meta["total_idx_cols"]], I16, kind="ExternalInput")
    dr_in = nc.dram_tensor("dstrel", [128, NT], F32, kind="ExternalInput")
    S_in = nc.dram_tensor("Ssel", [128, NB * G], F32, kind="ExternalInput")
    iota_in = nc.dram_tensor("iota", [128, 128], F32, kind="ExternalInput")
    id_in = nc.dram_tensor("ident", [128, 128], F32, kind="ExternalInput")
    W1aT_in = nc.dram_tensor("W1aT", [128, 256], F16, kind="ExternalInput")
    W1bT_in = nc.dram_tensor("W1bT", [128, 512], F16, kind="ExternalInput")
    W2aT_in = nc.dram_tensor("W2aT", [128, 512], F16, kind="ExternalInput")
    W2bT_in = nc.dram_tensor("W2bT", [128, 512], F16, kind="ExternalInput")
    bA1_in = nc.dram_tensor("bA1", [128, 2], F32, kind="ExternalInput")
    bA2_in = nc.dram_tensor("bA2", [128, 2], F32, kind="ExternalInput")
    bB1_in = nc.dram_tensor("bB1", [128, 256], F32, kind="ExternalInput")
    bB2_in = nc.dram_tensor("bB2", [128, 256], F32, kind="ExternalInput")
    pool_out = nc.dram_tensor("pool_out", [G, HID], F32, kind="ExternalOutput")

    # parity views of the x table: [NPAD/2, 2, IN_DIM]
    x2 = x_tab[:].rearrange("(a b) d -> a b d", b=2)

    with tile.TileContext(nc) as tc:
        with (
            tc.tile_pool(name="const", bufs=1) as constp,
            tc.tile_pool(name="h1own", bufs=1) as h1p,
            tc.tile_pool(name="dram", bufs=1, space="DRAM") as dramp,
        ):
            iota_t = constp.tile([128, 128], F32)
            nc.sync.dma_start(out=iota_t[:], in_=iota_in[:])
            ident_t = constp.tile([128, 128], F32)
            nc.sync.dma_start(out=ident_t[:], in_=id_in[:])
            dr_t = constp.tile([128, NT], F32)
            nc.sync.dma_start(out=dr_t[:], in_=dr_in[:])
            S_t = constp.tile([128, NB * G], F32)
            nc.sync.dma_start(out=S_t[:], in_=S_in[:])
            W1aT = constp.tile([128, 256], F16)
            nc.sync.dma_start(out=W1aT[:], in_=W1aT_in[:])
            W1bT = constp.tile([128, 512], F16)
            nc.sync.dma_start(out=W1bT[:], in_=W1bT_in[:])
            W2aT = constp.tile([128, 512], F16)
            nc.sync.dma_start(out=W2aT[:], in_=W2aT_in[:])
            W2bT = constp.tile([128, 512], F16)
            nc.sync.dma_start(out=W2bT[:], in_=W2bT_in[:])
            bA1 = constp.tile([128, 2], F32)
            nc.sync.dma_start(out=bA1[:], in_=bA1_in[:])
            bA2 = constp.tile([128, 2], F32)
            nc.sync.dma_start(out=bA2[:], in_=bA2_in[:])
            bB1 = constp.tile([128, 256], F32)
            nc.sync.dma_start(out=bB1[:], in_=bB1_in[:])
            bB2 = constp.tile([128, 256], F32)
            nc.sync.dma_start(out=bB2[:], in_=bB2_in[:])

            idx_all = constp.tile([128, meta["total_idx_cols"]], I16)
            nc.sync.dma_start(out=idx_all[:], in_=idx_in[:])

            h1own = h1p.tile([128, NB * HID], F32)    # own-shard h1, node-major

            h1_shard = dramp.tile([SH, HID], F16, name="h1_shard")
            h1_full = dramp.tile([NPAD, HID], F16, name="h1_full",
                                 addr_space="Shared")
            h12 = h1_full[:].rearrange("(a b) d -> a b d", b=2)

            def conv(ci):
                """Emit one GIN conv. ci = 0 or 1."""
                D = IN_DIM if ci == 0 else HID
                nch = D // 128                  # input-feature chunks
                gdt = F32 if ci == 0 else F16   # gathered dtype
                mdt = F32 if (ci == 0 and fp32_conv1_mm) else F16
                WaT, WbT = (W1aT, W1bT) if ci == 0 else (W2aT, W2bT)
                bA, bB = (bA1, bB1) if ci == 0 else (bA2, bB2)

                with (
                    tc.tile_pool(name=f"g{ci}", bufs=2) as gp,
                    tc.tile_pool(name=f"m{ci}", bufs=6) as mp,
                    tc.tile_pool(name=f"z{ci}", bufs=3) as zp,
                    tc.tile_pool(name=f"zt{ci}", bufs=3) as ztp,
                    tc.tile_pool(name=f"relu{ci}", bufs=3) as rp,
                    tc.tile_pool(name=f"h16_{ci}", bufs=3) as h16p,
                    tc.tile_pool(name=f"ps_agg{ci}", bufs=2, space="PSUM") as pagg,
                    tc.tile_pool(name=f"ps_tr{ci}", bufs=2, space="PSUM") as ptr,
                    tc.tile_pool(name=f"ps_a{ci}", bufs=2, space="PSUM") as pa,
                    tc.tile_pool(name=f"ps_b{ci}", bufs=1, space="PSUM") as pb,
                    tc.tile_pool(name=f"ps_pool{ci}", bufs=1, space="PSUM") as ppool,
                ):
                    if ci == 1:
                        pool_ps = ppool.tile([G, HID], F32, space="PSUM")
                    for gi, gblocks in enumerate(groups):
                        gb = len(gblocks)
                        ntile_g = gb * 2 * T_P
                        g = gp.tile([128, ntile_g, D], gdt, name="g")
                        # two parity gathers; even tiles occupy the first
                        # gb*T_P tile slots, odd the rest
                        for p in (0, 1):
                            _, _, off, cols = col_off[gi * 2 + p]
                            nidx = gb * cap
                            half_view = (x2 if ci == 0 else h12)[:, p, :]
                            nc.gpsimd.dma_gather(
                                g[:, p * gb * T_P:(p + 1) * gb * T_P, :],
                                half_view,
                                idx_all[:, off:off + cols],
                                nidx, nidx, D, elem_step=2 * D,
                                single_packet=False,
                            )
                        for bi, b in enumerate(gblocks):
                            agg = pagg.tile([128, D], F32, space="PSUM", name="agg")
                            for t in range(2 * T_P):
                                p, tp = (0, t) if t < T_P else (1, t - T_P)
                                gslot = p * gb * T_P + bi * T_P + tp
                                col = b * 2 * T_P + p * T_P + tp
                                m = mp.tile([128, 128], mdt, name="m")
                                nc.vector.tensor_scalar(
                                    out=m[:], in0=iota_t[:],
                                    scalar1=dr_t[:, col:col + 1], scalar2=None,
                                    op0=mybir.AluOpType.is_equal)
                                nc.tensor.matmul(
                                    agg[:], lhsT=m[:], rhs=g[:, gslot, :],
                                    start=(t == 0), stop=(t == 2 * T_P - 1))
                            # z = agg + own (node-major)
                            z = zp.tile([128, D], F32, name="z")
                            if ci == 0:
                                own = zp.tile([128, D], F32, name="own")
                                nc.sync.dma_start(
                                    out=own[:],
                                    in_=x_own[b * BLK:(b + 1) * BLK, :])
                                nc.vector.tensor_add(z[:], agg[:], own[:])
                            else:
                                nc.vector.tensor_add(
                                    z[:], agg[:],
                                    h1own[:, b * HID:(b + 1) * HID])
                            # transpose chunks -> fp16 z_t
                            zts = []
                            for k in range(nch):
                                trp = ptr.tile([128, 128], F32, space="PSUM",
                                               name="trp")
                                nc.tensor.transpose(
                                    out=trp[:], in_=z[:, k * 128:(k + 1) * 128],
                                    identity=ident_t[:])
                                zt = ztp.tile([128, 128], F16, name="zt")
                                nc.scalar.copy(zt[:], trp[:])
                                zts.append(zt)
                            # MLP A: relu(Wa @ z + bA)
                            relus = []
                            for hh in range(2):
                                pa_t = pa.tile([128, 128], F32, space="PSUM",
                                               name="pa")
                                for k in range(nch):
                                    lhs = WaT[:, k * 256 + hh * 128:
                                              k * 256 + (hh + 1) * 128]
                                    nc.tensor.matmul(
                                        pa_t[:], lhsT=lhs, rhs=zts[k][:],
                                        start=(k == 0), stop=(k == nch - 1))
                                relu = rp.tile([128, 128], F16, name="relu")
                                nc.scalar.activation(
                                    relu[:], pa_t[:], RELU,
                                    bias=bA[:, hh:hh + 1])
                                relus.append(relu)
                            # MLP B: h = Wb @ relu + bB   (node-major out)
                            pb_t = pb.tile([128, HID], F32, space="PSUM",
                                           name="pb")
                            for hh in range(2):
                                nc.tensor.matmul(
                                    pb_t[:], lhsT=relus[hh][:],
                                    rhs=WbT[:, hh * 256:(hh + 1) * 256],
                                    start=(hh == 0), stop=(hh == 1))
                            if ci == 0:
                                nc.vector.tensor_add(
                                    h1own[:, b * HID:(b + 1) * HID],
                                    pb_t[:], bB1[:])
                                h16 = h16p.tile([128, HID], F16, name="h16")
                                nc.scalar.copy(
                                    h16[:], h1own[:, b * HID:(b + 1) * HID])
                                nc.sync.dma_start(
                                    out=h1_shard[b * BLK:(b + 1) * BLK, :],
                                    in_=h16[:])
                            else:
                                h2 = zp.tile([128, HID], F32, name="h2")
                                nc.vector.tensor_add(h2[:], pb_t[:], bB2[:])
                                nc.tensor.matmul(
                                    pool_ps[:],
                                    lhsT=S_t[:, b * G:(b + 1) * G],
                                    rhs=h2[:],
                                    start=(b == 0), stop=(b == NB - 1))
                    if ci == 1:
                        pool_sb = zp.tile([G, HID], F32, name="pool_sb")
                        nc.vector.tensor_copy(out=pool_sb[:], in_=pool_ps[:])
                        nc.sync.dma_start(out=pool_out[:], in_=pool_sb[:])

            conv(0)
            nc.gpsimd.collective_compute(
                "AllGather", mybir.AluOpType.bypass,
                replica_groups=[list(range(NCORES))],
                ins=[h1_shard.opt()], outs=[h1_full.opt()])
            conv(1)

    nc.compile()
    return nc


def finish_host(pool_parts, batch, Wc, bc):
    """pool_parts: list of [64, 256] partial sums per core."""
    tot = np.sum(np.stack([p.astype(np.float64) for p in pool_parts]), axis=0)
    cnts = np.bincount(np.asarray(batch, dtype=np.int64), minlength=G).astype(np.float64)
    graph = (tot / np.maximum(cnts, 1.0)[:, None]).astype(np.float32)
    return graph @ Wc.T.astype(np.float32) + bc.astype(np.float32)


# ---------------------------------------------------------------------------
# Public entry point
# ---------------------------------------------------------------------------
from concourse.bass_utils import run_bass_kernel_spmd

_CACHE = {}


def _get_compiled(x, edge_index, batch, weights):
    in_maps, meta = preprocess(x, edge_index, batch, weights)
    key = (meta["T_P"], meta["total_idx_cols"])
    if key not in _CACHE:
        _CACHE[key] = build(meta)
    return _CACHE[key], in_maps, meta


def kernel(x, edge_index, batch, W1a, b1a, W1b, b1b, W2a, b2a, W2b, b2b, Wc, bc):
    x = np.asarray(x)
    edge_index = np.asarray(edge_index)
    batch = np.asarray(batch)
    weights = tuple(np.asarray(w) for w in (W1a, b1a, W1b, b1b, W2a, b2a, W2b, b2b))
    nc, in_maps, _ = _get_compiled(x, edge_index, batch, weights)
    res = run_bass_kernel_spmd(nc, in_maps, list(range(NCORES)))
    parts = [res.results[c]["pool_out"] for c in range(NCORES)]
    out = finish_host(parts, batch, np.asarray(Wc), np.asarray(bc))
    return out.astype(np.float32)
